# revision 21
# baseline (speedup 1.0000x reference)
"""Causal multi-head attention (B=4, S=2048, D=1024, H=16) on 8 NeuronCores.

Sharding: core c = (batch b = c//2, head-group hg = c%2). Each core computes
8 heads of one batch: QKV projection (bf16 matmuls), causal flash-style
attention (bf16 matmuls, exp-without-max softmax with a ones-column
denominator), and a row-parallel out-projection partial. Host sums the two
bf16 head-group partials per batch, adds bias, and transposes.

v2 schedule: attention emits score matmuls for kt-PAIRS back to back
(64-row-tile mode) then both attn@V groups (128 mode), halving PE
tile-mode switches. PSUM is segregated: sc pool (2 bufs x 2 banks),
ao (1 buf x 2 banks), work pool (2 bufs x 1 bank) for qkv/out-proj
fillers - so projection matmuls never serialize against the exp.
ao is evacuated to SBUF by the Scalar engine (frees the single ao
buffer fast); diag masks run on GpSimd; warm-up matmuls + a dummy exp
during the DMA prologue pre-warm the PE clock and the ACT exp table.
"""
import numpy as np
from contextlib import ExitStack

import ml_dtypes

B, S, D, H = 4, 2048, 1024, 16
HD = 64            # head dim
HPC = 8            # heads per core
F = HPC * HD       # 512 features per head-group
QT = 512           # q tile (free dim)
NQI = S // QT      # 4
NKT = S // 128     # 16
NDK = D // 128     # 8 contraction tiles for projections
C = HD + 1
SCALE = HD ** -0.5
DEBUG_TAPS = False

_CACHE = {}


def _build():
    import concourse.bacc as bacc
    import concourse.tile as tile
    import concourse.mybir as mybir

    f32 = mybir.dt.float32
    bf16 = mybir.dt.bfloat16
    EXP = mybir.ActivationFunctionType.Exp

    nc = bacc.Bacc("TRN2", target_bir_lowering=False, debug=False)
    xT = nc.dram_tensor("xT", [D, S], bf16, kind="ExternalInput").ap()
    w_sl = nc.dram_tensor("w_sl", [D, 3 * F], bf16, kind="ExternalInput").ap()
    wo_sl = nc.dram_tensor("wo_sl", [F, D], bf16, kind="ExternalInput").ap()
    mask2 = nc.dram_tensor("mask2", [128, 256], bf16, kind="ExternalInput").ap()
    out = nc.dram_tensor("out", [D, S], bf16, kind="ExternalOutput").ap()
    dbg = {}
    if DEBUG_TAPS:
        dbg["aoc"] = nc.dram_tensor("dbg_aoc", [C, 2 * QT], f32,
                                    kind="ExternalOutput").ap()
        dbg["srow"] = nc.dram_tensor("dbg_srow", [1, 2 * QT], f32,
                                     kind="ExternalOutput").ap()
        dbg["rb"] = nc.dram_tensor("dbg_rb", [HD, 2 * QT], f32,
                                   kind="ExternalOutput").ap()
        dbg["am"] = nc.dram_tensor("dbg_am", [128, QT], bf16,
                                   kind="ExternalOutput").ap()

    with tile.TileContext(nc) as tc:
        with ExitStack() as ctx:
            misc = ctx.enter_context(tc.tile_pool(name="misc", bufs=1))
            pqk = ctx.enter_context(tc.tile_pool(name="pqk", bufs=1))
            pv = ctx.enter_context(tc.tile_pool(name="pv", bufs=1))
            patt = ctx.enter_context(tc.tile_pool(name="patt", bufs=16))
            pP = ctx.enter_context(tc.tile_pool(name="pP", bufs=4))
            paoc = ctx.enter_context(tc.tile_pool(name="paoc", bufs=2))
            pr = ctx.enter_context(tc.tile_pool(name="pr", bufs=2))
            prr = ctx.enter_context(tc.tile_pool(name="prr", bufs=2))
            pwo = ctx.enter_context(tc.tile_pool(name="pwo", bufs=1))
            pxw = ctx.enter_context(tc.tile_pool(name="pxw", bufs=1))
            pstg = ctx.enter_context(tc.tile_pool(name="pstg", bufs=2))

            # PSUM: sc 2x2 banks + ao 1x2 banks + work 2x1 bank = 8 banks
            psc = ctx.enter_context(
                tc.tile_pool(name="psc", bufs=2, space="PSUM"))
            pao = ctx.enter_context(
                tc.tile_pool(name="pao", bufs=1, space="PSUM"))
            pwk = ctx.enter_context(
                tc.tile_pool(name="pwk", bufs=2, space="PSUM"))

            # ---- input DMAs, ordered for earliest compute start ----
            # qk proj is the attention critical path: wq first on sync.
            # x split by kk across scalar/gpsimd queues (parallel desc-gen).
            mask_sb = misc.tile([128, 256], bf16, name="mask_sb", tag="mask")
            nc.sync.dma_start(mask_sb[:], mask2)

            x_t = [pxw.tile([128, S], bf16, name=f"x{kk}", tag=f"x{kk}")
                   for kk in range(NDK)]
            wq_t = []
            wv_t = []
            for kk in range(NDK):
                r0 = slice(kk * 128, (kk + 1) * 128)
                wq = pxw.tile([128, 2 * F], bf16, name=f"wq{kk}",
                              tag=f"wq{kk}")
                nc.sync.dma_start(wq[:], w_sl[r0, 0:2 * F])
                wq_t.append(wq)
            for cs in (slice(0, QT), slice(QT, 2 * QT), slice(2 * QT, 4 * QT)):
                for kk in range(NDK):
                    r0 = slice(kk * 128, (kk + 1) * 128)
                    eng = nc.scalar if kk < 4 else nc.gpsimd
                    eng.dma_start(x_t[kk][:, cs], xT[r0, cs])
            for kk in range(NDK):
                r0 = slice(kk * 128, (kk + 1) * 128)
                wv = pxw.tile([128, F], bf16, name=f"wv{kk}", tag=f"wv{kk}")
                nc.sync.dma_start(wv[:], w_sl[r0, 2 * F:3 * F])
                wv_t.append(wv)
            wo_t = [pwo.tile([128, D], bf16, name=f"wo{g}", tag=f"wo{g}")
                    for g in range(4)]
            for g in range(4):
                nc.gpsimd.dma_start(wo_t[g][:],
                                    wo_sl[g * 128:(g + 1) * 128, :])

            # ---- PE warm-up scratch (HAM un-throttle filler) ----
            wsc = misc.tile([128, 128], bf16, name="wsc", tag="wsc")
            nc.vector.memset(wsc[:], 0.0)

            # ---- SBUF destinations for q/k/v ----
            q_sb = [pqk.tile([128, S], bf16, name=f"q{g}", tag=f"q{g}")
                    for g in range(4)]
            k_sb = [pqk.tile([128, S], bf16, name=f"k{g}", tag=f"k{g}")
                    for g in range(4)]
            v_sb = [pv.tile([128, HPC * C], bf16, name=f"v{t}",
                            tag=f"v{t}") for t in range(NKT)]

            att_m = {}

            def v_sub(tt):
                """V for token block tt (128 tokens), all 8 heads."""
                ps = pwk.tile([128, F], f32, name=f"pv{tt}", tag="wk")
                for kk in range(NDK):
                    nc.tensor.matmul(
                        ps[:], x_t[kk][:, tt * 128:(tt + 1) * 128],
                        wv_t[kk][:],
                        start=(kk == 0), stop=(kk == NDK - 1))
                vv = v_sb[tt].rearrange("p (h c) -> p h c", h=HPC)
                pp = ps.rearrange("p (h c) -> p h c", h=HPC)
                nc.vector.tensor_copy(vv[:, :, 0:HD], pp[:])
                nc.vector.memset(vv[:, :, HD:HD + 1], 1.0)

            def qk_sub(g, part, tq):
                """q or k features [g*128, g*128+128) for token group tq."""
                dest = q_sb if part == 0 else k_sb
                fcol = part * F + g * 128
                ts = slice(tq * QT, (tq + 1) * QT)
                ps = pwk.tile([128, QT], f32, name=f"pq{part}{g}{tq}",
                              tag="wk")
                for kk in range(NDK):
                    nc.tensor.matmul(
                        ps[:], wq_t[kk][:, fcol:fcol + 128],
                        x_t[kk][:, ts],
                        start=(kk == 0), stop=(kk == NDK - 1))
                nc.vector.tensor_copy(dest[g][:, ts], ps[:])

            s2_cur = {}
            s3_t = {}

            def _op_out(qi, dt):
                """Stage chunked output DMAs after stripes 0-3 and 4-7."""
                if dt == 3 or dt == 7:
                    lo = dt - 3
                    dst = out[lo * 128:(lo + 4) * 128,
                              qi * QT:(qi + 1) * QT].rearrange(
                        "(t p) c -> p t c", p=128)
                    nc.sync.dma_start(dst, s2_cur[qi][:, lo:lo + 4, :])

            def op_sub(qi, dt):
                """Out-projection columns [dt*128, dt*128+128) for q tile."""
                dcol = slice(dt * 128, dt * 128 + 128)
                ps = pwk.tile([128, QT], f32, name=f"op{dt}{qi}", tag="wk")
                for pg in range(4):
                    nc.tensor.matmul(
                        ps[:], wo_t[pg][:, dcol], att_m[(pg, qi)][:],
                        start=(pg == 0), stop=(pg == 3))
                if dt == 0:
                    s2_cur[qi] = pstg.tile([128, 8, QT], bf16,
                                           name=f"s2{qi}", tag="s2")
                nc.vector.tensor_copy(s2_cur[qi][:, dt, :], ps[:])
                _op_out(qi, dt)

            def op3_partial(dt):
                """Pairs 0-2 of the qi=3 out-projection, run during pair-3
                attention; staged to SBUF f32."""
                dcol = slice(dt * 128, dt * 128 + 128)
                ps = pwk.tile([128, QT], f32, name=f"o3p{dt}", tag="wk")
                for pg in range(3):
                    nc.tensor.matmul(
                        ps[:], wo_t[pg][:, dcol], att_m[(pg, 3)][:],
                        start=(pg == 0), stop=(pg == 2))
                s3 = pstg.tile([128, QT], f32, name=f"s3{dt}", tag=f"s3{dt}",
                               bufs=1)
                s3_t[dt] = s3
                nc.vector.tensor_copy(s3[:], ps[:])

            def op3_final(dt):
                """Last-pair contribution + staged partial -> output."""
                dcol = slice(dt * 128, dt * 128 + 128)
                ps = pwk.tile([128, QT], f32, name=f"o3f{dt}", tag="wk")
                nc.tensor.matmul(ps[:], wo_t[3][:, dcol], att_m[(3, 3)][:],
                                 start=True, stop=True)
                if dt == 0:
                    s2_cur[3] = pstg.tile([128, 8, QT], bf16,
                                          name="s2q3", tag="s2")
                nc.vector.tensor_add(s2_cur[3][:, dt, :], ps[:], s3_t[dt][:])
                _op_out(3, dt)

            m3 = mask_sb.rearrange("p (h c) -> p h c", h=2)

            def attn_block(pg, qi, fillers):
                """Block-causal attention for head pair pg, q tile qi.
                Scores for kt pairs are emitted back to back (64-row mode),
                then both attn@V groups (128 mode), then filler units."""
                nkt = 4 * qi + 4
                qs = qi * QT
                he, ho = 2 * pg, 2 * pg + 1
                nktp = nkt // 2
                # split fillers across the ktp slots
                # contiguous split preserves intra-list dependency order
                fsplit = [[] for _ in range(nktp)]
                nfl = len(fillers)
                for i, fn in enumerate(fillers):
                    fsplit[i * nktp // max(nfl, 1)].append(fn)
                ao = pao.tile([C, 2, QT], f32, name=f"ao{pg}{qi}", tag="ao")
                for ktp in range(nktp):
                    group = []
                    for j in (0, 1):
                        kt = 2 * ktp + j
                        d = kt - 4 * qi
                        n0 = 0 if d < 0 else 128 * d
                        kcol = slice(kt * 128, kt * 128 + 128)
                        sc = psc.tile([128, 2, QT], f32,
                                      name=f"sc{pg}{qi}{kt}", tag="sc")
                        nc.tensor.matmul(
                            sc[:, 0, n0:QT], k_sb[pg][0:64, kcol],
                            q_sb[pg][0:64, qs + n0:qs + QT],
                            start=True, stop=True)
                        nc.tensor.matmul(
                            sc[:, 1, n0:QT], k_sb[pg][64:128, kcol],
                            q_sb[pg][64:128, qs + n0:qs + QT],
                            start=True, stop=True)
                        group.append((sc, kt, d, n0))
                    pts = []
                    for sc, kt, d, n0 in group:
                        pt = pP.tile([128, 2, QT], bf16,
                                     name=f"pt{pg}{qi}{kt}", tag="P")
                        nc.scalar.activation(pt[:, :, n0:QT], sc[:, :, n0:QT],
                                             EXP, scale=SCALE)
                        if d >= 0:
                            nc.vector.tensor_mul(pt[:, :, n0:n0 + 128],
                                                 pt[:, :, n0:n0 + 128],
                                                 m3[:])
                        pts.append((pt, kt, n0))
                    for pt, kt, n0 in pts:
                        st = (kt == 0)
                        sp = (kt == nkt - 1)
                        vv = v_sb[kt].rearrange("p (h c) -> p h c", h=HPC)
                        nc.tensor.matmul(ao[:, 0, n0:QT], vv[:, he, :],
                                         pt[:, 0, n0:QT], start=st, stop=sp)
                        nc.tensor.matmul(ao[:, 1, n0:QT], vv[:, ho, :],
                                         pt[:, 1, n0:QT], start=st, stop=sp)
                    for fn in fsplit[ktp]:
                        fn()

                # normalize: evacuate ao on ACT, 1/rowsum, broadcast, scale.
                # Last block skips the ACT staging hop (shorter tail chain).
                last = (pg == 3 and qi == NQI - 1)
                if last:
                    src = ao
                else:
                    aoc = paoc.tile([C, 2, QT], f32, name=f"aoc{pg}{qi}",
                                    tag="aoc")
                    nc.scalar.copy(aoc[:], ao[:])
                    src = aoc
                # custom-DVE recip misreads nonzero base partitions on HW:
                # stage den into a partition-0 tile first
                srow0 = prr.tile([1, 2 * QT], f32, name=f"s0{pg}{qi}",
                                 tag="sr0")
                nc.vector.tensor_copy(
                    srow0[:], src[HD:HD + 1, :, :].rearrange("p h c -> p (h c)"))
                srow = prr.tile([1, 2 * QT], f32, name=f"sr{pg}{qi}",
                                tag="sr")
                nc.vector.reciprocal_approx_fast(srow[:], srow0[:])
                rb = pr.tile([HD, 2 * QT], f32, name=f"rb{pg}{qi}", tag="r")
                nc.gpsimd.partition_broadcast(rb[:], srow[:], channels=HD)
                am = patt.tile([128, QT], bf16, name=f"am{pg}{qi}", tag="am")
                att_m[(pg, qi)] = am
                nc.vector.tensor_mul(am[0:64, :], src[0:HD, 0, :],
                                     rb[:, 0:QT])
                nc.vector.tensor_mul(am[64:128, :], src[0:HD, 1, :],
                                     rb[:, QT:2 * QT])
                if DEBUG_TAPS and pg == 0 and qi == 0:
                    nc.sync.dma_start(dbg["aoc"],
                                      aoc.rearrange("p h c -> p (h c)"))
                    nc.sync.dma_start(dbg["srow"], srow[:])
                    nc.sync.dma_start(dbg["rb"], rb[:])
                    nc.sync.dma_start(dbg["am"], am[:])

            # ---- emission schedule ----
            # minimal prologue: pair-0 qk + v tokens 0:256, then warm-up
            # matmuls as scheduler-placed filler for early DMA stalls
            qk_sub(0, 0, 0)
            qk_sub(0, 1, 0)
            v_sub(0)
            v_sub(1)
            for i in range(44):
                ps = pwk.tile([128, 128], f32, name=f"wu{i}", tag="wk")
                nc.tensor.matmul(ps[:, 0:128], wsc[:], wsc[:],
                                 start=True, stop=True)

            def QK(g, p, t):
                return lambda: qk_sub(g, p, t)

            def VS(tt):
                return lambda: v_sub(tt)

            def O3P(dt):
                return lambda: op3_partial(dt)

            FILL = {
                (0, 0): [VS(2), VS(3), QK(0, 0, 1), QK(0, 1, 1),
                         VS(4), VS(5), VS(6), VS(7)],
                (0, 1): [QK(0, 0, 2), QK(0, 1, 2),
                         VS(8), VS(9), VS(10), VS(11)],
                (0, 2): [QK(0, 0, 3), QK(0, 1, 3),
                         VS(12), VS(13), VS(14), VS(15)],
                (0, 3): [QK(1, 0, 0), QK(1, 1, 0), QK(1, 0, 1), QK(1, 1, 1)],
                (1, 0): [QK(1, 0, 2), QK(1, 1, 2)],
                (1, 1): [QK(1, 0, 3), QK(1, 1, 3)],
                (1, 2): [QK(2, 0, 0), QK(2, 1, 0)],
                (1, 3): [QK(2, 0, 1), QK(2, 1, 1)],
                (2, 0): [QK(2, 0, 2), QK(2, 1, 2)],
                (2, 1): [QK(2, 0, 3), QK(2, 1, 3)],
                (2, 2): [QK(3, 0, 0), QK(3, 1, 0)],
                (2, 3): [QK(3, 0, 1), QK(3, 1, 1)],
                (3, 0): [QK(3, 0, 2), QK(3, 1, 2), O3P(0), O3P(1), O3P(2)],
                (3, 1): [QK(3, 0, 3), QK(3, 1, 3), O3P(3), O3P(4)],
                (3, 2): [O3P(5), O3P(6), O3P(7)],
                (3, 3): [],
            }
            for pg in range(4):
                for qi in range(NQI):
                    attn_block(pg, qi, FILL[(pg, qi)])
                    if pg == 3:
                        if qi < 3:
                            for dt in range(8):
                                op_sub(qi, dt)
                        else:
                            for dt in range(8):
                                op3_final(dt)

    nc.compile()
    return nc


def _get_nc():
    if "nc" not in _CACHE:
        _CACHE["nc"] = _build()
    return _CACHE["nc"]


def _prep_inputs(x, w_qkv, w_out, b_out):
    """Build the 8 per-core input maps (all payloads bf16)."""
    bf = ml_dtypes.bfloat16
    x = np.asarray(x, dtype=np.float32)
    w_qkv = np.asarray(w_qkv, dtype=np.float32)
    w_out = np.asarray(w_out, dtype=np.float32)

    tri = np.triu(np.ones((128, 128), dtype=np.float32))
    mask2 = np.tile(tri, (1, 2)).astype(bf)

    in_maps = []
    for c in range(8):
        b, hg = c // 2, c % 2
        cols = hg * F
        w_cat = np.concatenate([
            w_qkv[:, cols:cols + F],
            w_qkv[:, D + cols:D + cols + F],
            w_qkv[:, 2 * D + cols:2 * D + cols + F],
        ], axis=1)
        in_maps.append({
            "xT": np.ascontiguousarray(x[b].T).astype(bf),
            "w_sl": np.ascontiguousarray(w_cat).astype(bf),
            "wo_sl": np.ascontiguousarray(w_out[cols:cols + F, :]).astype(bf),
            "mask2": mask2,
        })
    return in_maps


def _run(inputs, trace=False):
    from concourse.bass_utils import run_bass_kernel_spmd

    nc = _get_nc()
    in_maps = _prep_inputs(**inputs)
    res = run_bass_kernel_spmd(nc, in_maps, core_ids=list(range(8)),
                               trace=trace)
    b_out = np.asarray(inputs["b_out"], dtype=np.float32)
    outs = []
    for b in range(B):
        o = (res.results[2 * b]["out"].astype(np.float32)
             + res.results[2 * b + 1]["out"].astype(np.float32))
        outs.append(o.T + b_out)
    full = np.stack(outs).astype(np.float32)
    return full, res


def kernel(x, w_qkv, w_out, b_out):
    full, _ = _run({"x": x, "w_qkv": w_qkv, "w_out": w_out, "b_out": b_out})
    return full


# revision 26
# speedup vs baseline: 1.0296x; 1.0296x over previous
"""Causal multi-head attention (B=4, S=2048, D=1024, H=16) on 8 NeuronCores.

Sharding: core c = (batch b = c//2, head-group hg = c%2). Each core computes
8 heads of one batch: QKV projection (bf16 matmuls), causal flash-style
attention (bf16 matmuls, exp-without-max softmax with a ones-column
denominator), and a row-parallel out-projection partial. Host sums the two
bf16 head-group partials per batch, adds bias, and transposes.

v2 schedule: attention emits score matmuls for kt-PAIRS back to back
(64-row-tile mode) then both attn@V groups (128 mode), halving PE
tile-mode switches. PSUM is segregated: sc pool (2 bufs x 2 banks),
ao (1 buf x 2 banks), work pool (2 bufs x 1 bank) for qkv/out-proj
fillers - so projection matmuls never serialize against the exp.
ao is evacuated to SBUF by the Scalar engine (frees the single ao
buffer fast); diag masks run on GpSimd; warm-up matmuls + a dummy exp
during the DMA prologue pre-warm the PE clock and the ACT exp table.
"""
import numpy as np
from contextlib import ExitStack

import ml_dtypes

B, S, D, H = 4, 2048, 1024, 16
HD = 64            # head dim
HPC = 8            # heads per core
F = HPC * HD       # 512 features per head-group
QT = 512           # q tile (free dim)
NQI = S // QT      # 4
NKT = S // 128     # 16
NDK = D // 128     # 8 contraction tiles for projections
C = HD + 1
SCALE = HD ** -0.5
DEBUG_TAPS = False

_CACHE = {}


def _build():
    import concourse.bacc as bacc
    import concourse.tile as tile
    import concourse.mybir as mybir

    f32 = mybir.dt.float32
    bf16 = mybir.dt.bfloat16
    EXP = mybir.ActivationFunctionType.Exp

    nc = bacc.Bacc("TRN2", target_bir_lowering=False, debug=False)
    xT = nc.dram_tensor("xT", [D, S], bf16, kind="ExternalInput").ap()
    w_sl = nc.dram_tensor("w_sl", [D, 3 * F], bf16, kind="ExternalInput").ap()
    wo_sl = nc.dram_tensor("wo_sl", [F, D], bf16, kind="ExternalInput").ap()
    mask2 = nc.dram_tensor("mask2", [128, 256], bf16, kind="ExternalInput").ap()
    out = nc.dram_tensor("out", [D, S], bf16, kind="ExternalOutput").ap()
    dbg = {}
    if DEBUG_TAPS:
        dbg["aoc"] = nc.dram_tensor("dbg_aoc", [C, 2 * QT], f32,
                                    kind="ExternalOutput").ap()
        dbg["srow"] = nc.dram_tensor("dbg_srow", [1, 2 * QT], f32,
                                     kind="ExternalOutput").ap()
        dbg["rb"] = nc.dram_tensor("dbg_rb", [HD, 2 * QT], f32,
                                   kind="ExternalOutput").ap()
        dbg["am"] = nc.dram_tensor("dbg_am", [128, QT], bf16,
                                   kind="ExternalOutput").ap()

    with tile.TileContext(nc) as tc:
        with ExitStack() as ctx:
            misc = ctx.enter_context(tc.tile_pool(name="misc", bufs=1))
            pqk = ctx.enter_context(tc.tile_pool(name="pqk", bufs=1))
            pv = ctx.enter_context(tc.tile_pool(name="pv", bufs=1))
            patt = ctx.enter_context(tc.tile_pool(name="patt", bufs=16))
            pP = ctx.enter_context(tc.tile_pool(name="pP", bufs=4))
            paoc = ctx.enter_context(tc.tile_pool(name="paoc", bufs=2))
            pr = ctx.enter_context(tc.tile_pool(name="pr", bufs=2))
            prr = ctx.enter_context(tc.tile_pool(name="prr", bufs=2))
            pwo = ctx.enter_context(tc.tile_pool(name="pwo", bufs=1))
            pxw = ctx.enter_context(tc.tile_pool(name="pxw", bufs=1))
            pstg = ctx.enter_context(tc.tile_pool(name="pstg", bufs=2))

            # PSUM: sc 2x2 banks + ao 1x2 banks + work 2x1 bank = 8 banks
            psc = ctx.enter_context(
                tc.tile_pool(name="psc", bufs=2, space="PSUM"))
            pao = ctx.enter_context(
                tc.tile_pool(name="pao", bufs=1, space="PSUM"))
            pwk = ctx.enter_context(
                tc.tile_pool(name="pwk", bufs=2, space="PSUM"))

            # ---- input DMAs ----
            # Contiguous full-row loads only: strided column-chunk loads pay
            # the 1KB-line HBM penalty (measured ~3-10x slower). x and w
            # trickle in kk by kk; projection matmuls chase the arrivals.
            mask_sb = misc.tile([128, 256], bf16, name="mask_sb", tag="mask")
            nc.sync.dma_start(mask_sb[:], mask2)

            x_t = [pxw.tile([128, S], bf16, name=f"x{kk}", tag=f"x{kk}")
                   for kk in range(NDK)]
            w_t = []
            for kk in range(NDK):
                r0 = slice(kk * 128, (kk + 1) * 128)
                w = pxw.tile([128, 3 * F], bf16, name=f"w{kk}", tag=f"w{kk}")
                w_t.append(w)
            for kk in range(NDK):
                r0 = slice(kk * 128, (kk + 1) * 128)
                eng = nc.scalar if kk % 2 == 0 else nc.gpsimd
                eng.dma_start(x_t[kk][:, :], xT[r0, :])
                nc.sync.dma_start(w_t[kk][:], w_sl[r0, :])
            wo_t = [pwo.tile([128, D], bf16, name=f"wo{g}", tag=f"wo{g}")
                    for g in range(4)]

            # ---- PE warm-up scratch (HAM un-throttle filler) ----
            wsc = misc.tile([128, 128], bf16, name="wsc", tag="wsc")
            nc.vector.memset(wsc[:], 0.0)

            # ---- SBUF destinations for q/k/v ----
            q_sb = [pqk.tile([128, S], bf16, name=f"q{g}", tag=f"q{g}")
                    for g in range(4)]
            k_sb = [pqk.tile([128, S], bf16, name=f"k{g}", tag=f"k{g}")
                    for g in range(4)]
            v_sb = [pv.tile([128, HPC * C], bf16, name=f"v{t}",
                            tag=f"v{t}") for t in range(NKT)]

            att_m = {}

            def v_sub(tt):
                """V for token block tt (128 tokens), all 8 heads."""
                ps = pwk.tile([128, F], f32, name=f"pv{tt}", tag="wk")
                for kk in range(NDK):
                    nc.tensor.matmul(
                        ps[:], x_t[kk][:, tt * 128:(tt + 1) * 128],
                        w_t[kk][:, 2 * F:3 * F],
                        start=(kk == 0), stop=(kk == NDK - 1))
                vv = v_sb[tt].rearrange("p (h c) -> p h c", h=HPC)
                pp = ps.rearrange("p (h c) -> p h c", h=HPC)
                nc.vector.tensor_copy(vv[:, :, 0:HD], pp[:])
                nc.vector.memset(vv[:, :, HD:HD + 1], 1.0)

            def qk_sub(g, part, tq):
                """q or k features [g*128, g*128+128) for token group tq."""
                dest = q_sb if part == 0 else k_sb
                fcol = part * F + g * 128
                ts = slice(tq * QT, (tq + 1) * QT)
                ps = pwk.tile([128, QT], f32, name=f"pq{part}{g}{tq}",
                              tag="wk")
                for kk in range(NDK):
                    nc.tensor.matmul(
                        ps[:], w_t[kk][:, fcol:fcol + 128],
                        x_t[kk][:, ts],
                        start=(kk == 0), stop=(kk == NDK - 1))
                nc.vector.tensor_copy(dest[g][:, ts], ps[:])

            s2_cur = {}
            s3_t = {}

            def _op_out(qi, dt):
                """Stage chunked output DMAs after stripes 0-3 and 4-7."""
                if dt == 3 or dt == 7:
                    lo = dt - 3
                    dst = out[lo * 128:(lo + 4) * 128,
                              qi * QT:(qi + 1) * QT].rearrange(
                        "(t p) c -> p t c", p=128)
                    nc.sync.dma_start(dst, s2_cur[qi][:, lo:lo + 4, :])

            def op_sub(qi, dt):
                """Out-projection columns [dt*128, dt*128+128) for q tile."""
                dcol = slice(dt * 128, dt * 128 + 128)
                ps = pwk.tile([128, QT], f32, name=f"op{dt}{qi}", tag="wk")
                for pg in range(4):
                    nc.tensor.matmul(
                        ps[:], wo_t[pg][:, dcol], att_m[(pg, qi)][:],
                        start=(pg == 0), stop=(pg == 3))
                if dt == 0:
                    s2_cur[qi] = pstg.tile([128, 8, QT], bf16,
                                           name=f"s2{qi}", tag="s2")
                nc.vector.tensor_copy(s2_cur[qi][:, dt, :], ps[:])
                _op_out(qi, dt)

            def op3_partial(dt):
                """Pairs 0-2 of the qi=3 out-projection, run during pair-3
                attention; staged to SBUF f32."""
                dcol = slice(dt * 128, dt * 128 + 128)
                ps = pwk.tile([128, QT], f32, name=f"o3p{dt}", tag="wk")
                for pg in range(3):
                    nc.tensor.matmul(
                        ps[:], wo_t[pg][:, dcol], att_m[(pg, 3)][:],
                        start=(pg == 0), stop=(pg == 2))
                s3 = pstg.tile([128, QT], f32, name=f"s3{dt}", tag=f"s3{dt}",
                               bufs=1)
                s3_t[dt] = s3
                nc.vector.tensor_copy(s3[:], ps[:])

            def op3_final(dt):
                """Last-pair contribution + staged partial -> output."""
                dcol = slice(dt * 128, dt * 128 + 128)
                ps = pwk.tile([128, QT], f32, name=f"o3f{dt}", tag="wk")
                nc.tensor.matmul(ps[:], wo_t[3][:, dcol], att_m[(3, 3)][:],
                                 start=True, stop=True)
                if dt == 0:
                    s2_cur[3] = pstg.tile([128, 8, QT], bf16,
                                          name="s2q3", tag="s2")
                nc.vector.tensor_add(s2_cur[3][:, dt, :], ps[:], s3_t[dt][:])
                _op_out(3, dt)

            m3 = mask_sb.rearrange("p (h c) -> p h c", h=2)

            def attn_block(pg, qi, fillers):
                """Block-causal attention for head pair pg, q tile qi.
                Scores for kt pairs are emitted back to back (64-row mode),
                then both attn@V groups (128 mode), then filler units."""
                nkt = 4 * qi + 4
                qs = qi * QT
                he, ho = 2 * pg, 2 * pg + 1
                nktp = nkt // 2
                # split fillers across the ktp slots
                # contiguous split preserves intra-list dependency order
                fsplit = [[] for _ in range(nktp)]
                nfl = len(fillers)
                for i, fn in enumerate(fillers):
                    fsplit[i * nktp // max(nfl, 1)].append(fn)
                ao = pao.tile([C, 2, QT], f32, name=f"ao{pg}{qi}", tag="ao")
                for ktp in range(nktp):
                    group = []
                    for j in (0, 1):
                        kt = 2 * ktp + j
                        d = kt - 4 * qi
                        n0 = 0 if d < 0 else 128 * d
                        kcol = slice(kt * 128, kt * 128 + 128)
                        sc = psc.tile([128, 2, QT], f32,
                                      name=f"sc{pg}{qi}{kt}", tag="sc")
                        nc.tensor.matmul(
                            sc[:, 0, n0:QT], k_sb[pg][0:64, kcol],
                            q_sb[pg][0:64, qs + n0:qs + QT],
                            start=True, stop=True)
                        nc.tensor.matmul(
                            sc[:, 1, n0:QT], k_sb[pg][64:128, kcol],
                            q_sb[pg][64:128, qs + n0:qs + QT],
                            start=True, stop=True)
                        group.append((sc, kt, d, n0))
                    pts = []
                    for sc, kt, d, n0 in group:
                        pt = pP.tile([128, 2, QT], bf16,
                                     name=f"pt{pg}{qi}{kt}", tag="P")
                        nc.scalar.activation(pt[:, :, n0:QT], sc[:, :, n0:QT],
                                             EXP, scale=SCALE)
                        if d >= 0:
                            nc.vector.tensor_mul(pt[:, :, n0:n0 + 128],
                                                 pt[:, :, n0:n0 + 128],
                                                 m3[:])
                        pts.append((pt, kt, n0))
                    for pt, kt, n0 in pts:
                        st = (kt == 0)
                        sp = (kt == nkt - 1)
                        vv = v_sb[kt].rearrange("p (h c) -> p h c", h=HPC)
                        nc.tensor.matmul(ao[:, 0, n0:QT], vv[:, he, :],
                                         pt[:, 0, n0:QT], start=st, stop=sp)
                        nc.tensor.matmul(ao[:, 1, n0:QT], vv[:, ho, :],
                                         pt[:, 1, n0:QT], start=st, stop=sp)
                    for fn in fsplit[ktp]:
                        fn()

                # normalize: evacuate ao on ACT, 1/rowsum, broadcast, scale.
                # Last block skips the ACT staging hop (shorter tail chain).
                last = (pg == 3 and qi == NQI - 1)
                if last:
                    src = ao
                else:
                    aoc = paoc.tile([C, 2, QT], f32, name=f"aoc{pg}{qi}",
                                    tag="aoc")
                    nc.scalar.copy(aoc[:], ao[:])
                    src = aoc
                # custom-DVE recip misreads nonzero base partitions on HW:
                # stage den into a partition-0 tile first
                srow0 = prr.tile([1, 2 * QT], f32, name=f"s0{pg}{qi}",
                                 tag="sr0")
                nc.vector.tensor_copy(
                    srow0[:], src[HD:HD + 1, :, :].rearrange("p h c -> p (h c)"))
                srow = prr.tile([1, 2 * QT], f32, name=f"sr{pg}{qi}",
                                tag="sr")
                nc.vector.reciprocal_approx_fast(srow[:], srow0[:])
                rb = pr.tile([HD, 2 * QT], f32, name=f"rb{pg}{qi}", tag="r")
                nc.gpsimd.partition_broadcast(rb[:], srow[:], channels=HD)
                am = patt.tile([128, QT], bf16, name=f"am{pg}{qi}", tag="am")
                att_m[(pg, qi)] = am
                nc.vector.tensor_mul(am[0:64, :], src[0:HD, 0, :],
                                     rb[:, 0:QT])
                nc.vector.tensor_mul(am[64:128, :], src[0:HD, 1, :],
                                     rb[:, QT:2 * QT])
                if DEBUG_TAPS and pg == 0 and qi == 0:
                    nc.sync.dma_start(dbg["aoc"],
                                      aoc.rearrange("p h c -> p (h c)"))
                    nc.sync.dma_start(dbg["srow"], srow[:])
                    nc.sync.dma_start(dbg["rb"], rb[:])
                    nc.sync.dma_start(dbg["am"], am[:])

            # ---- emission schedule ----
            # warm-up matmuls first (HAM un-throttle while DMAs land),
            # then pair-0 qk + v chasing the kk arrival order
            for i in range(60):
                ps = pwk.tile([128, 128], f32, name=f"wu{i}", tag="wk")
                nc.tensor.matmul(ps[:, 0:128], wsc[:], wsc[:],
                                 start=True, stop=True)
            qk_sub(0, 0, 0)
            qk_sub(0, 1, 0)
            v_sub(0)
            v_sub(1)

            def QK(g, p, t):
                return lambda: qk_sub(g, p, t)

            def VS(tt):
                return lambda: v_sub(tt)

            def O3P(dt):
                return lambda: op3_partial(dt)

            FILL = {
                (0, 0): [VS(2), VS(3), QK(0, 0, 1), QK(0, 1, 1),
                         VS(4), VS(5), VS(6), VS(7)],
                (0, 1): [QK(0, 0, 2), QK(0, 1, 2),
                         VS(8), VS(9), VS(10), VS(11)],
                (0, 2): [QK(0, 0, 3), QK(0, 1, 3),
                         VS(12), VS(13), VS(14), VS(15)],
                (0, 3): [QK(1, 0, 0), QK(1, 1, 0), QK(1, 0, 1), QK(1, 1, 1)],
                (1, 0): [QK(1, 0, 2), QK(1, 1, 2)],
                (1, 1): [QK(1, 0, 3), QK(1, 1, 3)],
                (1, 2): [QK(2, 0, 0), QK(2, 1, 0)],
                (1, 3): [QK(2, 0, 1), QK(2, 1, 1)],
                (2, 0): [QK(2, 0, 2), QK(2, 1, 2)],
                (2, 1): [QK(2, 0, 3), QK(2, 1, 3)],
                (2, 2): [QK(3, 0, 0), QK(3, 1, 0)],
                (2, 3): [QK(3, 0, 1), QK(3, 1, 1)],
                (3, 0): [QK(3, 0, 2), QK(3, 1, 2), O3P(0), O3P(1)],
                (3, 1): [QK(3, 0, 3), QK(3, 1, 3), O3P(2), O3P(3)],
                (3, 2): [O3P(4), O3P(5)],
                (3, 3): [],
            }

            def WO():
                for g in range(4):
                    nc.gpsimd.dma_start(wo_t[g][:],
                                        wo_sl[g * 128:(g + 1) * 128, :])

            FILL[(0, 1)].append(WO)
            for pg in range(4):
                for qi in range(NQI):
                    attn_block(pg, qi, FILL[(pg, qi)])
                    if pg == 3:
                        if qi < 3:
                            for dt in range(8):
                                op_sub(qi, dt)
                        else:
                            # fill the last normalize-chain latency with the
                            # remaining partial out-projections
                            op3_partial(6)
                            op3_partial(7)
                            for dt in range(8):
                                op3_final(dt)

    nc.compile()
    return nc


def _get_nc():
    if "nc" not in _CACHE:
        _CACHE["nc"] = _build()
    return _CACHE["nc"]


def _prep_inputs(x, w_qkv, w_out, b_out):
    """Build the 8 per-core input maps (all payloads bf16)."""
    bf = ml_dtypes.bfloat16
    x = np.asarray(x, dtype=np.float32)
    w_qkv = np.asarray(w_qkv, dtype=np.float32)
    w_out = np.asarray(w_out, dtype=np.float32)

    tri = np.triu(np.ones((128, 128), dtype=np.float32))
    mask2 = np.tile(tri, (1, 2)).astype(bf)

    in_maps = []
    for c in range(8):
        b, hg = c // 2, c % 2
        cols = hg * F
        w_cat = np.concatenate([
            w_qkv[:, cols:cols + F],
            w_qkv[:, D + cols:D + cols + F],
            w_qkv[:, 2 * D + cols:2 * D + cols + F],
        ], axis=1)
        in_maps.append({
            "xT": np.ascontiguousarray(x[b].T).astype(bf),
            "w_sl": np.ascontiguousarray(w_cat).astype(bf),
            "wo_sl": np.ascontiguousarray(w_out[cols:cols + F, :]).astype(bf),
            "mask2": mask2,
        })
    return in_maps


def _run(inputs, trace=False):
    from concourse.bass_utils import run_bass_kernel_spmd

    nc = _get_nc()
    in_maps = _prep_inputs(**inputs)
    res = run_bass_kernel_spmd(nc, in_maps, core_ids=list(range(8)),
                               trace=trace)
    b_out = np.asarray(inputs["b_out"], dtype=np.float32)
    outs = []
    for b in range(B):
        o = (res.results[2 * b]["out"].astype(np.float32)
             + res.results[2 * b + 1]["out"].astype(np.float32))
        outs.append(o.T + b_out)
    full = np.stack(outs).astype(np.float32)
    return full, res


def kernel(x, w_qkv, w_out, b_out):
    full, _ = _run({"x": x, "w_qkv": w_qkv, "w_out": w_out, "b_out": b_out})
    return full


# revision 32
# speedup vs baseline: 1.0311x; 1.0015x over previous
"""Causal multi-head attention (B=4, S=2048, D=1024, H=16) on 8 NeuronCores.

Sharding: core c = (batch b = c//2, head-group hg = c%2). Each core computes
8 heads of one batch: QKV projection (bf16 matmuls), causal flash-style
attention (bf16 matmuls, exp-without-max softmax with a ones-column
denominator), and a row-parallel out-projection partial. Host sums the two
bf16 head-group partials per batch, adds bias, and transposes.

v2 schedule: attention emits score matmuls for kt-PAIRS back to back
(64-row-tile mode) then both attn@V groups (128 mode), halving PE
tile-mode switches. PSUM is segregated: sc pool (2 bufs x 2 banks),
ao (1 buf x 2 banks), work pool (2 bufs x 1 bank) for qkv/out-proj
fillers - so projection matmuls never serialize against the exp.
ao is evacuated to SBUF by the Scalar engine (frees the single ao
buffer fast); diag masks run on GpSimd; warm-up matmuls + a dummy exp
during the DMA prologue pre-warm the PE clock and the ACT exp table.
"""
import numpy as np
from contextlib import ExitStack

import ml_dtypes

B, S, D, H = 4, 2048, 1024, 16
HD = 64            # head dim
HPC = 8            # heads per core
F = HPC * HD       # 512 features per head-group
QT = 512           # q tile (free dim)
NQI = S // QT      # 4
NKT = S // 128     # 16
NDK = D // 128     # 8 contraction tiles for projections
C = HD + 1
SCALE = HD ** -0.5
DEBUG_TAPS = False

_CACHE = {}


def _build():
    import concourse.bacc as bacc
    import concourse.tile as tile
    import concourse.mybir as mybir

    f32 = mybir.dt.float32
    bf16 = mybir.dt.bfloat16
    EXP = mybir.ActivationFunctionType.Exp

    nc = bacc.Bacc("TRN2", target_bir_lowering=False, debug=False)
    xT = nc.dram_tensor("xT", [D, S], bf16, kind="ExternalInput").ap()
    w_sl = nc.dram_tensor("w_sl", [D, 3 * F], bf16, kind="ExternalInput").ap()
    wo_sl = nc.dram_tensor("wo_sl", [F, D], bf16, kind="ExternalInput").ap()
    mask2 = nc.dram_tensor("mask2", [128, 256], bf16, kind="ExternalInput").ap()
    out = nc.dram_tensor("out", [D, S], bf16, kind="ExternalOutput").ap()
    dbg = {}
    if DEBUG_TAPS:
        dbg["aoc"] = nc.dram_tensor("dbg_aoc", [C, 2 * QT], f32,
                                    kind="ExternalOutput").ap()
        dbg["srow"] = nc.dram_tensor("dbg_srow", [1, 2 * QT], f32,
                                     kind="ExternalOutput").ap()
        dbg["rb"] = nc.dram_tensor("dbg_rb", [HD, 2 * QT], f32,
                                   kind="ExternalOutput").ap()
        dbg["am"] = nc.dram_tensor("dbg_am", [128, QT], bf16,
                                   kind="ExternalOutput").ap()

    with tile.TileContext(nc) as tc:
        with ExitStack() as ctx:
            misc = ctx.enter_context(tc.tile_pool(name="misc", bufs=1))
            pqk = ctx.enter_context(tc.tile_pool(name="pqk", bufs=1))
            pv = ctx.enter_context(tc.tile_pool(name="pv", bufs=1))
            patt = ctx.enter_context(tc.tile_pool(name="patt", bufs=16))
            pP = ctx.enter_context(tc.tile_pool(name="pP", bufs=4))
            paoc = ctx.enter_context(tc.tile_pool(name="paoc", bufs=2))
            pr = ctx.enter_context(tc.tile_pool(name="pr", bufs=2))
            prr = ctx.enter_context(tc.tile_pool(name="prr", bufs=2))
            pwo = ctx.enter_context(tc.tile_pool(name="pwo", bufs=1))
            pxw = ctx.enter_context(tc.tile_pool(name="pxw", bufs=1))
            pstg = ctx.enter_context(tc.tile_pool(name="pstg", bufs=2))

            # PSUM: sc 2x2 banks + ao 1x2 banks + work 2x1 bank = 8 banks
            psc = ctx.enter_context(
                tc.tile_pool(name="psc", bufs=2, space="PSUM"))
            pao = ctx.enter_context(
                tc.tile_pool(name="pao", bufs=1, space="PSUM"))
            pwk = ctx.enter_context(
                tc.tile_pool(name="pwk", bufs=2, space="PSUM"))

            # ---- input DMAs ----
            # Contiguous full-row loads only: strided column-chunk loads pay
            # the 1KB-line HBM penalty (measured ~3-10x slower). x and w
            # trickle in kk by kk; projection matmuls chase the arrivals.
            mask_sb = misc.tile([128, 256], bf16, name="mask_sb", tag="mask")
            nc.sync.dma_start(mask_sb[:], mask2)

            x_t = [pxw.tile([128, S], bf16, name=f"x{kk}", tag=f"x{kk}")
                   for kk in range(NDK)]
            w_t = []
            wv_t = []
            for kk in range(NDK):
                w = pxw.tile([128, 2 * F], bf16, name=f"w{kk}", tag=f"w{kk}")
                w_t.append(w)
                wv = pxw.tile([128, F], bf16, name=f"wv{kk}", tag=f"wv{kk}")
                wv_t.append(wv)
            for kk in range(NDK):
                r0 = slice(kk * 128, (kk + 1) * 128)
                eng = nc.scalar if kk % 2 == 0 else nc.gpsimd
                eng.dma_start(x_t[kk][:, :], xT[r0, :])
                nc.sync.dma_start(w_t[kk][:], w_sl[r0, 0:2 * F])
            # wv (non-critical) issued after x on the scalar ring
            for kk in range(NDK):
                r0 = slice(kk * 128, (kk + 1) * 128)
                nc.scalar.dma_start(wv_t[kk][:], w_sl[r0, 2 * F:3 * F])
            wo_t = [pwo.tile([128, D], bf16, name=f"wo{g}", tag=f"wo{g}")
                    for g in range(4)]

            # ---- PE warm-up scratch (HAM un-throttle filler) ----
            wsc = misc.tile([128, 128], bf16, name="wsc", tag="wsc")
            nc.vector.memset(wsc[:], 0.0)

            # ---- SBUF destinations for q/k/v ----
            q_sb = [pqk.tile([128, S], bf16, name=f"q{g}", tag=f"q{g}")
                    for g in range(4)]
            k_sb = [pqk.tile([128, S], bf16, name=f"k{g}", tag=f"k{g}")
                    for g in range(4)]
            v_sb = [pv.tile([128, HPC * C], bf16, name=f"v{t}",
                            tag=f"v{t}") for t in range(NKT)]

            att_m = {}

            def v_sub(tt):
                """V for token block tt (128 tokens), all 8 heads."""
                ps = pwk.tile([128, F], f32, name=f"pv{tt}", tag="wk")
                for kk in range(NDK):
                    nc.tensor.matmul(
                        ps[:], x_t[kk][:, tt * 128:(tt + 1) * 128],
                        wv_t[kk][:],
                        start=(kk == 0), stop=(kk == NDK - 1))
                vv = v_sb[tt].rearrange("p (h c) -> p h c", h=HPC)
                pp = ps.rearrange("p (h c) -> p h c", h=HPC)
                nc.vector.tensor_copy(vv[:, :, 0:HD], pp[:])
                nc.vector.memset(vv[:, :, HD:HD + 1], 1.0)

            def qk_sub(g, part, tq):
                """q or k features [g*128, g*128+128) for token group tq."""
                dest = q_sb if part == 0 else k_sb
                fcol = part * F + g * 128
                ts = slice(tq * QT, (tq + 1) * QT)
                ps = pwk.tile([128, QT], f32, name=f"pq{part}{g}{tq}",
                              tag="wk")
                for kk in range(NDK):
                    nc.tensor.matmul(
                        ps[:], w_t[kk][:, fcol:fcol + 128],
                        x_t[kk][:, ts],
                        start=(kk == 0), stop=(kk == NDK - 1))
                nc.vector.tensor_copy(dest[g][:, ts], ps[:])

            s2_cur = {}
            s3_t = {}

            def _op_out(qi, dt):
                """Stage chunked output DMAs after stripes 0-3 and 4-7."""
                if dt == 3 or dt == 7:
                    lo = dt - 3
                    dst = out[lo * 128:(lo + 4) * 128,
                              qi * QT:(qi + 1) * QT].rearrange(
                        "(t p) c -> p t c", p=128)
                    nc.sync.dma_start(dst, s2_cur[qi][:, lo:lo + 4, :])

            def op_sub(qi, dt):
                """Out-projection columns [dt*128, dt*128+128) for q tile."""
                dcol = slice(dt * 128, dt * 128 + 128)
                ps = pwk.tile([128, QT], f32, name=f"op{dt}{qi}", tag="wk")
                for pg in range(4):
                    nc.tensor.matmul(
                        ps[:], wo_t[pg][:, dcol], att_m[(pg, qi)][:],
                        start=(pg == 0), stop=(pg == 3))
                if dt == 0:
                    s2_cur[qi] = pstg.tile([128, 8, QT], bf16,
                                           name=f"s2{qi}", tag="s2")
                nc.vector.tensor_copy(s2_cur[qi][:, dt, :], ps[:])
                _op_out(qi, dt)

            def op3_partial(dt):
                """Pairs 0-2 of the LAST-processed q tile's out-projection
                (qi=0), run during pair-3 attention; staged to SBUF f32."""
                dcol = slice(dt * 128, dt * 128 + 128)
                ps = pwk.tile([128, QT], f32, name=f"o3p{dt}", tag="wk")
                for pg in range(3):
                    nc.tensor.matmul(
                        ps[:], wo_t[pg][:, dcol], att_m[(pg, 0)][:],
                        start=(pg == 0), stop=(pg == 2))
                s3 = pstg.tile([128, QT], f32, name=f"s3{dt}", tag=f"s3{dt}",
                               bufs=1)
                s3_t[dt] = s3
                nc.vector.tensor_copy(s3[:], ps[:])

            def op3_final(dt):
                """Last-pair contribution + staged partial -> output."""
                dcol = slice(dt * 128, dt * 128 + 128)
                ps = pwk.tile([128, QT], f32, name=f"o3f{dt}", tag="wk")
                nc.tensor.matmul(ps[:], wo_t[3][:, dcol], att_m[(3, 0)][:],
                                 start=True, stop=True)
                if dt == 0:
                    s2_cur[0] = pstg.tile([128, 8, QT], bf16,
                                          name="s2q0", tag="s2")
                nc.vector.tensor_add(s2_cur[0][:, dt, :], ps[:], s3_t[dt][:])
                _op_out(0, dt)

            m3 = mask_sb.rearrange("p (h c) -> p h c", h=2)

            def attn_block(pg, qi, fillers):
                """Block-causal attention for head pair pg, q tile qi.
                Scores for kt pairs are emitted back to back (64-row mode),
                then both attn@V groups (128 mode), then filler units."""
                nkt = 4 * qi + 4
                qs = qi * QT
                he, ho = 2 * pg, 2 * pg + 1
                nktp = nkt // 2
                # split fillers across the ktp slots
                # contiguous split preserves intra-list dependency order
                fsplit = [[] for _ in range(nktp)]
                nfl = len(fillers)
                for i, fn in enumerate(fillers):
                    fsplit[i * nktp // max(nfl, 1)].append(fn)
                ao = pao.tile([C, 2, QT], f32, name=f"ao{pg}{qi}", tag="ao")
                for ktp in range(nktp):
                    group = []
                    for j in (0, 1):
                        kt = 2 * ktp + j
                        d = kt - 4 * qi
                        n0 = 0 if d < 0 else 128 * d
                        kcol = slice(kt * 128, kt * 128 + 128)
                        sc = psc.tile([128, 2, QT], f32,
                                      name=f"sc{pg}{qi}{kt}", tag="sc")
                        nc.tensor.matmul(
                            sc[:, 0, n0:QT], k_sb[pg][0:64, kcol],
                            q_sb[pg][0:64, qs + n0:qs + QT],
                            start=True, stop=True)
                        nc.tensor.matmul(
                            sc[:, 1, n0:QT], k_sb[pg][64:128, kcol],
                            q_sb[pg][64:128, qs + n0:qs + QT],
                            start=True, stop=True)
                        group.append((sc, kt, d, n0))
                    pts = []
                    for sc, kt, d, n0 in group:
                        pt = pP.tile([128, 2, QT], bf16,
                                     name=f"pt{pg}{qi}{kt}", tag="P")
                        nc.scalar.activation(pt[:, :, n0:QT], sc[:, :, n0:QT],
                                             EXP, scale=SCALE)
                        if d >= 0:
                            nc.vector.tensor_mul(pt[:, :, n0:n0 + 128],
                                                 pt[:, :, n0:n0 + 128],
                                                 m3[:])
                        pts.append((pt, kt, n0))
                    for pt, kt, n0 in pts:
                        st = (kt == 0)
                        sp = (kt == nkt - 1)
                        vv = v_sb[kt].rearrange("p (h c) -> p h c", h=HPC)
                        nc.tensor.matmul(ao[:, 0, n0:QT], vv[:, he, :],
                                         pt[:, 0, n0:QT], start=st, stop=sp)
                        nc.tensor.matmul(ao[:, 1, n0:QT], vv[:, ho, :],
                                         pt[:, 1, n0:QT], start=st, stop=sp)
                    for fn in fsplit[ktp]:
                        fn()

                # normalize: evacuate ao on ACT, 1/rowsum, broadcast, scale.
                # Last block skips the ACT staging hop (shorter tail chain).
                last = (pg == 3 and qi == 0)
                if last:
                    src = ao
                else:
                    aoc = paoc.tile([C, 2, QT], f32, name=f"aoc{pg}{qi}",
                                    tag="aoc")
                    nc.scalar.copy(aoc[:], ao[:])
                    src = aoc
                # custom-DVE recip misreads nonzero base partitions on HW:
                # stage den into a partition-0 tile first
                srow0 = prr.tile([1, 2 * QT], f32, name=f"s0{pg}{qi}",
                                 tag="sr0")
                nc.vector.tensor_copy(
                    srow0[:], src[HD:HD + 1, :, :].rearrange("p h c -> p (h c)"))
                srow = prr.tile([1, 2 * QT], f32, name=f"sr{pg}{qi}",
                                tag="sr")
                nc.vector.reciprocal_approx_fast(srow[:], srow0[:])
                rb = pr.tile([HD, 2 * QT], f32, name=f"rb{pg}{qi}", tag="r")
                nc.gpsimd.partition_broadcast(rb[:], srow[:], channels=HD)
                am = patt.tile([128, QT], bf16, name=f"am{pg}{qi}", tag="am")
                att_m[(pg, qi)] = am
                nc.vector.tensor_mul(am[0:64, :], src[0:HD, 0, :],
                                     rb[:, 0:QT])
                nc.vector.tensor_mul(am[64:128, :], src[0:HD, 1, :],
                                     rb[:, QT:2 * QT])
                if DEBUG_TAPS and pg == 0 and qi == 0:
                    nc.sync.dma_start(dbg["aoc"],
                                      aoc.rearrange("p h c -> p (h c)"))
                    nc.sync.dma_start(dbg["srow"], srow[:])
                    nc.sync.dma_start(dbg["rb"], rb[:])
                    nc.sync.dma_start(dbg["am"], am[:])

            # ---- emission schedule ----
            # warm-up matmuls first (HAM un-throttle while DMAs land),
            # then all of pair-0 qk + v chasing the kk arrival order
            for i in range(60):
                ps = pwk.tile([128, 128], f32, name=f"wu{i}", tag="wk")
                nc.tensor.matmul(ps[:, 0:128], wsc[:], wsc[:],
                                 start=True, stop=True)
            for tq in range(4):
                qk_sub(0, 0, tq)
                qk_sub(0, 1, tq)
            for tt in range(8):
                v_sub(tt)

            def QK(g, p, t):
                return lambda: qk_sub(g, p, t)

            def VS(tt):
                return lambda: v_sub(tt)

            def O3P(dt):
                return lambda: op3_partial(dt)

            def WO():
                for g in range(4):
                    nc.gpsimd.dma_start(wo_t[g][:],
                                        wo_sl[g * 128:(g + 1) * 128, :])

            FILL = {
                (0, 0): [VS(8), VS(9), VS(10), VS(11)],
                (0, 1): [VS(12), VS(13), VS(14), VS(15), WO],
                (0, 2): [QK(1, 0, 0), QK(1, 1, 0)],
                (0, 3): [QK(1, 0, 1), QK(1, 1, 1), QK(1, 0, 2), QK(1, 1, 2)],
                (1, 0): [QK(1, 0, 3), QK(1, 1, 3)],
                (1, 1): [QK(2, 0, 0), QK(2, 1, 0)],
                (1, 2): [QK(2, 0, 1), QK(2, 1, 1)],
                (1, 3): [QK(2, 0, 2), QK(2, 1, 2)],
                (2, 0): [QK(2, 0, 3), QK(2, 1, 3)],
                (2, 1): [QK(3, 0, 0), QK(3, 1, 0)],
                (2, 2): [QK(3, 0, 1), QK(3, 1, 1)],
                (2, 3): [QK(3, 0, 2), QK(3, 1, 2)],
                (3, 1): [QK(3, 0, 3), QK(3, 1, 3), O3P(0), O3P(1)],
                (3, 2): [O3P(2), O3P(3), O3P(4)],
                (3, 3): [O3P(5), O3P(6), O3P(7)],
                (3, 0): [],
            }
            for pg in range(4):
                # pair 3 runs qi=0 last: its out-projection is pg-split so
                # only one matmul per dt stripe trails the final normalize
                qis = [1, 2, 3, 0] if pg == 3 else list(range(NQI))
                for qi in qis:
                    attn_block(pg, qi, FILL[(pg, qi)])
                    if pg == 3:
                        if qi != 0:
                            for dt in range(8):
                                op_sub(qi, dt)
                        else:
                            for dt in range(8):
                                op3_final(dt)

    nc.compile()
    return nc


def _get_nc():
    if "nc" not in _CACHE:
        _CACHE["nc"] = _build()
    return _CACHE["nc"]


def _prep_inputs(x, w_qkv, w_out, b_out):
    """Build the 8 per-core input maps (all payloads bf16)."""
    bf = ml_dtypes.bfloat16
    x = np.asarray(x, dtype=np.float32)
    w_qkv = np.asarray(w_qkv, dtype=np.float32)
    w_out = np.asarray(w_out, dtype=np.float32)

    tri = np.triu(np.ones((128, 128), dtype=np.float32))
    mask2 = np.tile(tri, (1, 2)).astype(bf)

    in_maps = []
    for c in range(8):
        b, hg = c // 2, c % 2
        cols = hg * F
        w_cat = np.concatenate([
            w_qkv[:, cols:cols + F],
            w_qkv[:, D + cols:D + cols + F],
            w_qkv[:, 2 * D + cols:2 * D + cols + F],
        ], axis=1)
        in_maps.append({
            "xT": np.ascontiguousarray(x[b].T).astype(bf),
            "w_sl": np.ascontiguousarray(w_cat).astype(bf),
            "wo_sl": np.ascontiguousarray(w_out[cols:cols + F, :]).astype(bf),
            "mask2": mask2,
        })
    return in_maps


def _run(inputs, trace=False):
    from concourse.bass_utils import run_bass_kernel_spmd

    nc = _get_nc()
    in_maps = _prep_inputs(**inputs)
    res = run_bass_kernel_spmd(nc, in_maps, core_ids=list(range(8)),
                               trace=trace)
    b_out = np.asarray(inputs["b_out"], dtype=np.float32)
    outs = []
    for b in range(B):
        o = (res.results[2 * b]["out"].astype(np.float32)
             + res.results[2 * b + 1]["out"].astype(np.float32))
        outs.append(o.T + b_out)
    full = np.stack(outs).astype(np.float32)
    return full, res


def kernel(x, w_qkv, w_out, b_out):
    full, _ = _run({"x": x, "w_qkv": w_qkv, "w_out": w_out, "b_out": b_out})
    return full
